# revision 14
# baseline (speedup 1.0000x reference)
"""APPNP on 8 TRN2 NeuronCores.

Z0 = (h @ W1 + b1) @ W2 + b2            [N, C]
Zk = (1-a) * (adj @ Zk) + a * Z0        x K iterations
out = log_softmax(Zk, axis=1)

Sharding: row-parallel. Core r owns rows [r*S, (r+1)*S) of adj / h / Z.
adj is passed host-transposed per shard (adjt_r = adj[rS:(r+1)S, :].T,
shape [N, S], contiguous) so SBUF tiles load with the contraction axis
(source node s) on partitions naturally.

Per iteration on core r:
  Y^T[c, t] = sum_s  Zfull[s-tile]-as-lhsT  x  adjt[s-tile, t]   (PSUM)
  Znext = (1-a) * Y + a*Z0  via PE-transpose of Y^T + scalar_tensor_tensor
  AllGather Znext -> Zfull, split into nTB quarter-gathers pipelined
  against the next iteration's matmul sweep (sweep visits s-tiles in
  quarter order, so quarter q+1's gather overlaps quarter q's compute).
Final: row-wise log_softmax, DMA out shard [S, C].

dtype_mode: "f32" | "f32r" | "bf16" | "fp8e4" | "fp8e3" — storage/compute
dtype of adj tiles (Z tiles are bf16 for fp8 modes; PSUM always f32).
"""

import sys

import numpy as np

if "/opt/trn_rl_repo" not in sys.path:
    sys.path.insert(0, "/opt/trn_rl_repo")

N, F, HID, C = 16384, 512, 256, 64
ALPHA, KITERS = 0.1, 10
NCORES = 8

DTYPE_MODE = "fp8e3"


def build_graph(n=N, f=F, hid=HID, c=C, ncores=NCORES, k_iters=KITERS,
                dtype_mode=DTYPE_MODE, alpha=ALPHA):
    from concourse import bacc, mybir, tile
    from concourse import masks

    f32 = mybir.dt.float32
    f32r = mybir.dt.float32r
    bf16 = mybir.dt.bfloat16
    S = n // ncores              # shard rows per core
    nST = n // 128               # global s-tiles
    nTM = S // 128               # local 128-row tiles
    TBLK = min(512, S)           # moving free-dim per matmul
    nTB = S // TBLK              # psum blocks covering the shard rows
    nJQ = TBLK // 128            # local 128-row tiles per quarter
    NBLK = min(512, S)           # MLP stage-A moving block
    nNB = S // NBLK
    nKF = f // 128               # feature k-tiles
    nKH = hid // 128             # hidden k-tiles
    nHM = hid // 128             # hidden m-tiles (stage A outputs)
    rg = [list(range(ncores))]

    adj_dt = {"f32": f32, "f32r": f32r, "bf16": bf16,
              "fp8e4": mybir.dt.float8e4, "fp8e3": mybir.dt.float8e3}[dtype_mode]
    z_dt = {"f32": f32, "f32r": f32r, "bf16": bf16,
            "fp8e4": bf16, "fp8e3": bf16}[dtype_mode]
    ag_dt = bf16 if z_dt == bf16 else f32

    nc = bacc.Bacc(None, target_bir_lowering=False)

    adjt = nc.declare_dram_parameter("adjt", [n, S], adj_dt, False)
    ht = nc.declare_dram_parameter("ht", [f, S], f32, False)
    w1 = nc.declare_dram_parameter("w1", [f, hid], f32, False)
    b1 = nc.declare_dram_parameter("b1", [1, hid], f32, False)
    w2 = nc.declare_dram_parameter("w2", [hid, c], f32, False)
    b2 = nc.declare_dram_parameter("b2", [1, c], f32, False)
    out = nc.declare_dram_parameter("out", [S, c], f32, True)

    ag_space = "Shared" if ncores > 4 else "Local"
    agins = [nc.dram_tensor(f"agin{q}", [S // nTB, c], ag_dt)
             for q in range(nTB)]
    agouts = [nc.dram_tensor(f"agout{q}", [n // nTB, c], ag_dt,
                             addr_space=ag_space) for q in range(nTB)]

    # s-tile visit order: quarter q covers global tiles {nTM*r + nJQ*q + jl}
    def quarter_tiles(q):
        return [nTM * r + nJQ * q + jl
                for r in range(ncores) for jl in range(nJQ)]

    with tile.TileContext(nc) as tc:
        with (
            tc.tile_pool(name="const", bufs=1) as cpool,
            tc.tile_pool(name="zp", bufs=1) as zpool,
            tc.tile_pool(name="zc", bufs=2) as zcpool,
            tc.tile_pool(name="psum", bufs=2, space="PSUM") as psum,
            tc.tile_pool(name="psy", bufs=4, space="PSUM") as psy,
            tc.tile_pool(name="pst", bufs=2, space="PSUM") as pst,
        ):
            # ---- constants ----
            ident = cpool.tile([64, 64], f32, tag="ident")
            masks.make_identity(nc, ident[:])
            ones = cpool.tile([1, max(NBLK, 128)], f32, tag="ones")
            nc.gpsimd.memset(ones[:], 1.0)
            w1sb = cpool.tile([128, nKF, hid], f32, tag="w1")
            nc.sync.dma_start(out=w1sb[:], in_=w1.ap().rearrange("(a p) h -> p a h", p=128))
            w2sb = cpool.tile([128, nKH, c], f32, tag="w2")
            nc.sync.dma_start(out=w2sb[:], in_=w2.ap().rearrange("(a p) h -> p a h", p=128))
            b1sb = cpool.tile([1, hid], f32, tag="b1")
            nc.sync.dma_start(out=b1sb[:], in_=b1[:, :])
            b2sb = cpool.tile([1, c], f32, tag="b2")
            nc.sync.dma_start(out=b2sb[:], in_=b2[:, :])

            z0s = zpool.tile([128, nTM, c], f32, tag="z0s")      # alpha * Z0
            zfull = zpool.tile([128, nST, c], z_dt, tag="zfull")  # gathered Zk

            # quarter-q AllGather: zcur quarter -> zfull quarter columns
            agin_dma = nc.gpsimd if ag_dt != f32 else nc.sync
            zf_dma = nc.gpsimd if (z_dt != ag_dt) else nc.sync

            def issue_ag(q, zsrc):
                agin_dma.dma_start(
                    out=agins[q].ap().rearrange("(p j) c -> p j c", p=128),
                    in_=zsrc[:, nJQ * q:nJQ * (q + 1), :])
                nc.gpsimd.collective_compute(
                    "AllGather", mybir.AluOpType.bypass,
                    ins=[agins[q].ap().opt()], outs=[agouts[q].ap().opt()],
                    replica_groups=rg)
                zf_dma.dma_start(
                    out=zfull[:].rearrange(
                        "p (r q j) c -> p r q j c",
                        r=ncores, q=nTB)[:, :, q, :, :],
                    in_=agouts[q].ap().rearrange(
                        "(r p j) c -> p r j c", r=ncores, p=128))

            # ---- MLP ----
            with tc.tile_pool(name="mlp", bufs=1) as mpool:
                htsb = mpool.tile([128, nKF, S], f32, tag="ht")
                nc.sync.dma_start(out=htsb[:], in_=ht.ap().rearrange("(a p) s -> p a s", p=128))
                x1t = mpool.tile([128, nHM, S], f32, tag="x1t")
                for m in range(nHM):
                    for nb in range(nNB):
                        px = psum.tile([128, NBLK], f32, tag="mlp")
                        for kf in range(nKF):
                            nc.tensor.matmul(
                                px[:],
                                lhsT=w1sb[:, kf, m * 128:(m + 1) * 128],
                                rhs=htsb[:, kf, nb * NBLK:(nb + 1) * NBLK],
                                start=(kf == 0), stop=False)
                        nc.tensor.matmul(
                            px[:],
                            lhsT=b1sb[0:1, m * 128:(m + 1) * 128],
                            rhs=ones[0:1, 0:NBLK],
                            start=False, stop=True)
                        nc.vector.tensor_copy(x1t[:, m, nb * NBLK:(nb + 1) * NBLK], px[:])

                zcur = zcpool.tile([128, nTM, c], f32, tag="zcur")
                for tm in range(nTM):
                    pz = pst.tile([128, c], f32, tag="ptz")
                    for kh in range(nKH):
                        nc.tensor.matmul(
                            pz[:],
                            lhsT=x1t[:, kh, tm * 128:(tm + 1) * 128],
                            rhs=w2sb[:, kh, :],
                            start=(kh == 0), stop=False)
                    nc.tensor.matmul(
                        pz[:],
                        lhsT=ones[0:1, 0:128],
                        rhs=b2sb[0:1, :],
                        start=False, stop=True)
                    nc.vector.tensor_copy(zcur[:, tm, :], pz[:])
                    nc.scalar.mul(z0s[:, tm, :], pz[:], alpha)
                for q in range(nTB):
                    issue_ag(q, zcur)

            # ---- propagation ----
            with (
                tc.tile_pool(name="adj", bufs=10) as apool,
                tc.tile_pool(name="ys", bufs=4) as ypool,
            ):
                TP = min(2, nTB)          # t-blocks per column-pass
                nTP = nTB // TP
                at_dma = nc.gpsimd if adj_dt == f32r else nc.sync
                for it in range(k_iters):
                    pys = [psy.tile([64, TBLK], f32, tag="py", name=f"py{it}_{t}")
                           for t in range(nTB)]
                    znext = zcpool.tile([128, nTM, c], f32, tag="zcur")
                    last = it == k_iters - 1

                    def epilogue(t):
                        ysb = ypool.tile([64, TBLK], f32, tag="ysb",
                                         name=f"ysb{it}_{t}")
                        nc.vector.tensor_copy(ysb[:], pys[t][:])
                        for j in range(nJQ):
                            tm = t * nJQ + j
                            ptz = pst.tile([128, c], f32, tag="ptz",
                                           name=f"ptz{it}_{tm}")
                            nc.tensor.transpose(
                                ptz[:], ysb[0:64, j * 128:(j + 1) * 128], ident[:])
                            nc.vector.scalar_tensor_tensor(
                                out=znext[:, tm, :],
                                in0=ptz[:], scalar=1.0 - alpha, in1=z0s[:, tm, :],
                                op0=mybir.AluOpType.mult, op1=mybir.AluOpType.add)
                        if not last:
                            issue_ag(t, znext)

                    # 2D-blocked sweep: s-quarters outer (consumes incoming
                    # quarter-gathers progressively), t-pairs inner (banks of a
                    # pair finish early in the last s-quarter pass, so their
                    # outgoing gathers fire before the sweep ends).
                    for q_in in range(nTB):
                        stiles = quarter_tiles(q_in)
                        for tp in range(nTP):
                            for s in stiles:
                                at = apool.tile([128, TP * TBLK], adj_dt, tag="at")
                                at_dma.dma_start(
                                    out=at[:],
                                    in_=adjt[s * 128:(s + 1) * 128,
                                             tp * TP * TBLK:(tp + 1) * TP * TBLK])
                                for i in range(TP):
                                    t = tp * TP + i
                                    nc.tensor.matmul(
                                        pys[t][:],
                                        lhsT=zfull[:, s, :],
                                        rhs=at[:, i * TBLK:(i + 1) * TBLK],
                                        start=(q_in == 0 and s == stiles[0]),
                                        stop=(q_in == nTB - 1 and s == stiles[-1]))
                            if q_in == nTB - 1:
                                for i in range(TP):
                                    epilogue(tp * TP + i)
                    zcur = znext

            # ---- log_softmax rows of zcur ----
            with tc.tile_pool(name="sm", bufs=4) as spool:
                outsb = zpool.tile([128, nTM, c], f32, tag="outsb")
                for tm in range(nTM):
                    zt = zcur[:, tm, :]
                    negm = spool.tile([128, 1], f32, tag="negm")
                    nc.vector.tensor_reduce(
                        negm[:], zt, axis=mybir.AxisListType.X,
                        op=mybir.AluOpType.max, negate=True)
                    e = spool.tile([128, c], f32, tag="e")
                    ssum = spool.tile([128, 1], f32, tag="ssum")
                    nc.scalar.activation(
                        e[:], zt, mybir.ActivationFunctionType.Exp,
                        bias=negm[:], scale=1.0, accum_out=ssum[:])
                    nlse = spool.tile([128, 1], f32, tag="nlse")
                    nc.scalar.activation(
                        nlse[:], ssum[:], mybir.ActivationFunctionType.Ln)
                    nc.vector.tensor_scalar_mul(nlse[:], nlse[:], -1.0)
                    t1 = spool.tile([128, c], f32, tag="t1")
                    nc.scalar.activation(
                        t1[:], zt, mybir.ActivationFunctionType.Identity,
                        bias=negm[:])
                    nc.scalar.activation(
                        outsb[:, tm, :], t1[:],
                        mybir.ActivationFunctionType.Identity, bias=nlse[:])
                nc.sync.dma_start(
                    out=out.ap().rearrange("(j p) c -> p j c", p=128),
                    in_=outsb[:])

    nc.finalize()
    return nc


def make_in_maps(h, adj, W1, b1, W2, b2, n=N, ncores=NCORES,
                 dtype_mode=DTYPE_MODE):
    import ml_dtypes
    adj_np_dt = {"f32": np.float32, "f32r": np.float32,
                 "bf16": ml_dtypes.bfloat16,
                 "fp8e4": ml_dtypes.float8_e4m3,
                 "fp8e3": ml_dtypes.float8_e3m4}[dtype_mode]
    S = n // ncores
    in_maps = []
    for r in range(ncores):
        in_maps.append({
            "adjt": np.ascontiguousarray(
                adj[r * S:(r + 1) * S, :].T.astype(adj_np_dt)),
            "ht": np.ascontiguousarray(h[r * S:(r + 1) * S, :].T),
            "w1": np.ascontiguousarray(W1),
            "b1": np.ascontiguousarray(b1).reshape(1, -1),
            "w2": np.ascontiguousarray(W2),
            "b2": np.ascontiguousarray(b2).reshape(1, -1),
        })
    return in_maps


_RUN_KW = {}  # test harness can set {"trace": True, "tmpdir": ...}
_LAST_RESULT = [None]


def kernel(h, adj, W1, b1, W2, b2):
    from concourse.bass_utils import run_bass_kernel_spmd

    h = np.ascontiguousarray(h, dtype=np.float32)
    adj = np.ascontiguousarray(adj, dtype=np.float32)
    nc = build_graph()
    in_maps = make_in_maps(h, adj, W1, b1, W2, b2)
    res = run_bass_kernel_spmd(nc, in_maps, core_ids=list(range(NCORES)), **_RUN_KW)
    _LAST_RESULT[0] = res
    return np.concatenate([res.results[r]["out"] for r in range(NCORES)], axis=0)


# revision 15
# speedup vs baseline: 1.3193x; 1.3193x over previous
"""APPNP on 8 TRN2 NeuronCores.

Z0 = (h @ W1 + b1) @ W2 + b2            [N, C]
Zk = (1-a) * (adj @ Zk) + a * Z0        x K iterations
out = log_softmax(Zk, axis=1)

Sharding: row-parallel. Core r owns rows [r*S, (r+1)*S) of adj / h / Z.
adj is passed host-transposed per shard (adjt_r = adj[rS:(r+1)S, :].T,
shape [N, S], contiguous) so SBUF tiles load with the contraction axis
(source node s) on partitions naturally.

Per iteration on core r:
  Y^T[c, t] = sum_s  Zfull[s-tile]-as-lhsT  x  adjt[s-tile, t]   (PSUM)
  Znext = (1-a) * Y + a*Z0  via PE-transpose of Y^T + scalar_tensor_tensor
  AllGather Znext -> Zfull, split into nTB quarter-gathers pipelined
  against the next iteration's matmul sweep (sweep visits s-tiles in
  quarter order, so quarter q+1's gather overlaps quarter q's compute).
Final: row-wise log_softmax, DMA out shard [S, C].

dtype_mode: "f32" | "f32r" | "bf16" | "fp8e4" | "fp8e3" — storage/compute
dtype of adj tiles (Z tiles are bf16 for fp8 modes; PSUM always f32).
"""

import sys

import numpy as np

if "/opt/trn_rl_repo" not in sys.path:
    sys.path.insert(0, "/opt/trn_rl_repo")

N, F, HID, C = 16384, 512, 256, 64
ALPHA, KITERS = 0.1, 10
NCORES = 8

DTYPE_MODE = "fp8e3"


def build_graph(n=N, f=F, hid=HID, c=C, ncores=NCORES, k_iters=KITERS,
                dtype_mode=DTYPE_MODE, alpha=ALPHA):
    from concourse import bacc, mybir, tile
    from concourse import masks

    f32 = mybir.dt.float32
    f32r = mybir.dt.float32r
    bf16 = mybir.dt.bfloat16
    S = n // ncores              # shard rows per core
    nST = n // 128               # global s-tiles
    nTM = S // 128               # local 128-row tiles
    TBLK = min(512, S)           # moving free-dim per matmul
    nTB = S // TBLK              # psum blocks covering the shard rows
    nJQ = TBLK // 128            # local 128-row tiles per quarter
    NBLK = min(512, S)           # MLP stage-A moving block
    nNB = S // NBLK
    nKF = f // 128               # feature k-tiles
    nKH = hid // 128             # hidden k-tiles
    nHM = hid // 128             # hidden m-tiles (stage A outputs)
    rg = [list(range(ncores))]

    adj_dt = {"f32": f32, "f32r": f32r, "bf16": bf16,
              "fp8e4": mybir.dt.float8e4, "fp8e3": mybir.dt.float8e3}[dtype_mode]
    z_dt = {"f32": f32, "f32r": f32r, "bf16": bf16,
            "fp8e4": bf16, "fp8e3": bf16}[dtype_mode]
    ag_dt = bf16 if z_dt == bf16 else f32

    nc = bacc.Bacc(None, target_bir_lowering=False)

    adjt = nc.declare_dram_parameter("adjt", [n, S], adj_dt, False)
    ht = nc.declare_dram_parameter("ht", [f, S], f32, False)
    w1 = nc.declare_dram_parameter("w1", [f, hid], f32, False)
    b1 = nc.declare_dram_parameter("b1", [1, hid], f32, False)
    w2 = nc.declare_dram_parameter("w2", [hid, c], f32, False)
    b2 = nc.declare_dram_parameter("b2", [1, c], f32, False)
    out = nc.declare_dram_parameter("out", [S, c], f32, True)

    ag_space = "Shared" if ncores > 4 else "Local"
    agins = [nc.dram_tensor(f"agin{q}", [S // nTB, c], ag_dt)
             for q in range(nTB)]
    agouts = [nc.dram_tensor(f"agout{q}", [n // nTB, c], ag_dt,
                             addr_space=ag_space) for q in range(nTB)]

    # s-tile visit order: quarter q covers global tiles {nTM*r + nJQ*q + jl}
    def quarter_tiles(q):
        return [nTM * r + nJQ * q + jl
                for r in range(ncores) for jl in range(nJQ)]

    with tile.TileContext(nc) as tc:
        with (
            tc.tile_pool(name="const", bufs=1) as cpool,
            tc.tile_pool(name="zp", bufs=1) as zpool,
            tc.tile_pool(name="zc", bufs=2) as zcpool,
            tc.tile_pool(name="psum", bufs=2, space="PSUM") as psum,
            tc.tile_pool(name="psy", bufs=4, space="PSUM") as psy,
            tc.tile_pool(name="pst", bufs=2, space="PSUM") as pst,
        ):
            # ---- constants ----
            ident = cpool.tile([64, 64], f32, tag="ident")
            masks.make_identity(nc, ident[:])
            ones = cpool.tile([1, max(NBLK, 128)], f32, tag="ones")
            nc.gpsimd.memset(ones[:], 1.0)
            w1sb = cpool.tile([128, nKF, hid], f32, tag="w1")
            nc.sync.dma_start(out=w1sb[:], in_=w1.ap().rearrange("(a p) h -> p a h", p=128))
            w2sb = cpool.tile([128, nKH, c], f32, tag="w2")
            nc.sync.dma_start(out=w2sb[:], in_=w2.ap().rearrange("(a p) h -> p a h", p=128))
            b1sb = cpool.tile([1, hid], f32, tag="b1")
            nc.sync.dma_start(out=b1sb[:], in_=b1[:, :])
            b2sb = cpool.tile([1, c], f32, tag="b2")
            nc.sync.dma_start(out=b2sb[:], in_=b2[:, :])

            z0s = zpool.tile([128, nTM, c], f32, tag="z0s")      # alpha * Z0
            zfull = zpool.tile([128, nST, c], z_dt, tag="zfull")  # gathered Zk

            # quarter-q AllGather: zcur quarter -> zfull quarter columns
            agin_dma = nc.gpsimd if ag_dt != f32 else nc.sync
            zf_dma = nc.gpsimd if (z_dt != ag_dt) else nc.sync

            def issue_ag(q, zsrc):
                agin_dma.dma_start(
                    out=agins[q].ap().rearrange("(p j) c -> p j c", p=128),
                    in_=zsrc[:, nJQ * q:nJQ * (q + 1), :])
                nc.gpsimd.collective_compute(
                    "AllGather", mybir.AluOpType.bypass,
                    ins=[agins[q].ap().opt()], outs=[agouts[q].ap().opt()],
                    replica_groups=rg)
                zf_dma.dma_start(
                    out=zfull[:].rearrange(
                        "p (r q j) c -> p r q j c",
                        r=ncores, q=nTB)[:, :, q, :, :],
                    in_=agouts[q].ap().rearrange(
                        "(r p j) c -> p r j c", r=ncores, p=128))

            # ---- MLP ----
            with tc.tile_pool(name="mlp", bufs=1) as mpool:
                htsb = mpool.tile([128, nKF, S], f32, tag="ht")
                nc.sync.dma_start(out=htsb[:], in_=ht.ap().rearrange("(a p) s -> p a s", p=128))
                x1t = mpool.tile([128, nHM, S], f32, tag="x1t")
                for m in range(nHM):
                    for nb in range(nNB):
                        px = psum.tile([128, NBLK], f32, tag="mlp")
                        for kf in range(nKF):
                            nc.tensor.matmul(
                                px[:],
                                lhsT=w1sb[:, kf, m * 128:(m + 1) * 128],
                                rhs=htsb[:, kf, nb * NBLK:(nb + 1) * NBLK],
                                start=(kf == 0), stop=False)
                        nc.tensor.matmul(
                            px[:],
                            lhsT=b1sb[0:1, m * 128:(m + 1) * 128],
                            rhs=ones[0:1, 0:NBLK],
                            start=False, stop=True)
                        nc.vector.tensor_copy(x1t[:, m, nb * NBLK:(nb + 1) * NBLK], px[:])

                zcur = zcpool.tile([128, nTM, c], f32, tag="zcur")
                for tm in range(nTM):
                    pz = pst.tile([128, c], f32, tag="ptz")
                    for kh in range(nKH):
                        nc.tensor.matmul(
                            pz[:],
                            lhsT=x1t[:, kh, tm * 128:(tm + 1) * 128],
                            rhs=w2sb[:, kh, :],
                            start=(kh == 0), stop=False)
                    nc.tensor.matmul(
                        pz[:],
                        lhsT=ones[0:1, 0:128],
                        rhs=b2sb[0:1, :],
                        start=False, stop=True)
                    nc.vector.tensor_copy(zcur[:, tm, :], pz[:])
                    nc.scalar.mul(z0s[:, tm, :], pz[:], alpha)
                for q in range(nTB):
                    issue_ag(q, zcur)

            # ---- propagation ----
            with (
                tc.tile_pool(name="adj", bufs=10) as apool,
                tc.tile_pool(name="adj3", bufs=ncores * nJQ) as q3pool,
                tc.tile_pool(name="ys", bufs=4) as ypool,
            ):
                at_dma = nc.gpsimd if adj_dt == f32r else nc.sync
                nQ3 = ncores * nJQ        # tiles in the resident last quarter
                for it in range(k_iters):
                    pys = [psy.tile([64, TBLK], f32, tag="py", name=f"py{it}_{t}")
                           for t in range(nTB)]
                    znext = zcpool.tile([128, nTM, c], f32, tag="zcur")
                    last = it == k_iters - 1

                    def epilogue(t):
                        ysb = ypool.tile([64, TBLK], f32, tag="ysb",
                                         name=f"ysb{it}_{t}")
                        nc.vector.tensor_copy(ysb[:], pys[t][:])
                        for j in range(nJQ):
                            tm = t * nJQ + j
                            ptz = pst.tile([128, c], f32, tag="ptz",
                                           name=f"ptz{it}_{tm}")
                            nc.tensor.transpose(
                                ptz[:], ysb[0:64, j * 128:(j + 1) * 128], ident[:])
                            nc.vector.scalar_tensor_tensor(
                                out=znext[:, tm, :],
                                in0=ptz[:], scalar=1.0 - alpha, in1=z0s[:, tm, :],
                                op0=mybir.AluOpType.mult, op1=mybir.AluOpType.add)
                        if not last:
                            issue_ag(t, znext)

                    # Sweep in s-quarter order (consumes incoming quarter
                    # gathers progressively). The last quarter's adj tiles stay
                    # resident and its matmuls run t-major, so PSUM bank t
                    # completes early and its outgoing gather fires before the
                    # sweep ends.
                    for q_in in range(nTB - 1):
                        stiles = quarter_tiles(q_in)
                        for s in stiles:
                            at = apool.tile([128, S], adj_dt, tag="at")
                            at_dma.dma_start(
                                out=at[:], in_=adjt[s * 128:(s + 1) * 128, :])
                            for t in range(nTB):
                                nc.tensor.matmul(
                                    pys[t][:],
                                    lhsT=zfull[:, s, :],
                                    rhs=at[:, t * TBLK:(t + 1) * TBLK],
                                    start=(q_in == 0 and s == stiles[0]),
                                    stop=False)
                    stiles = quarter_tiles(nTB - 1)
                    at3s = []
                    for i, s in enumerate(stiles):
                        a = q3pool.tile([128, S], adj_dt, tag="at3",
                                        name=f"at3_{it}_{i}")
                        at_dma.dma_start(
                            out=a[:], in_=adjt[s * 128:(s + 1) * 128, :])
                        at3s.append(a)
                    for t in range(nTB):
                        for i, s in enumerate(stiles):
                            nc.tensor.matmul(
                                pys[t][:],
                                lhsT=zfull[:, s, :],
                                rhs=at3s[i][:, t * TBLK:(t + 1) * TBLK],
                                start=(nTB == 1 and t == 0 and s == stiles[0]),
                                stop=(s == stiles[-1]))
                        epilogue(t)
                    zcur = znext

            # ---- log_softmax rows of zcur ----
            with tc.tile_pool(name="sm", bufs=4) as spool:
                outsb = zpool.tile([128, nTM, c], f32, tag="outsb")
                for tm in range(nTM):
                    zt = zcur[:, tm, :]
                    negm = spool.tile([128, 1], f32, tag="negm")
                    nc.vector.tensor_reduce(
                        negm[:], zt, axis=mybir.AxisListType.X,
                        op=mybir.AluOpType.max, negate=True)
                    e = spool.tile([128, c], f32, tag="e")
                    ssum = spool.tile([128, 1], f32, tag="ssum")
                    nc.scalar.activation(
                        e[:], zt, mybir.ActivationFunctionType.Exp,
                        bias=negm[:], scale=1.0, accum_out=ssum[:])
                    nlse = spool.tile([128, 1], f32, tag="nlse")
                    nc.scalar.activation(
                        nlse[:], ssum[:], mybir.ActivationFunctionType.Ln)
                    nc.vector.tensor_scalar_mul(nlse[:], nlse[:], -1.0)
                    t1 = spool.tile([128, c], f32, tag="t1")
                    nc.scalar.activation(
                        t1[:], zt, mybir.ActivationFunctionType.Identity,
                        bias=negm[:])
                    nc.scalar.activation(
                        outsb[:, tm, :], t1[:],
                        mybir.ActivationFunctionType.Identity, bias=nlse[:])
                nc.sync.dma_start(
                    out=out.ap().rearrange("(j p) c -> p j c", p=128),
                    in_=outsb[:])

    nc.finalize()
    return nc


def make_in_maps(h, adj, W1, b1, W2, b2, n=N, ncores=NCORES,
                 dtype_mode=DTYPE_MODE):
    import ml_dtypes
    adj_np_dt = {"f32": np.float32, "f32r": np.float32,
                 "bf16": ml_dtypes.bfloat16,
                 "fp8e4": ml_dtypes.float8_e4m3,
                 "fp8e3": ml_dtypes.float8_e3m4}[dtype_mode]
    S = n // ncores
    in_maps = []
    for r in range(ncores):
        in_maps.append({
            "adjt": np.ascontiguousarray(
                adj[r * S:(r + 1) * S, :].T.astype(adj_np_dt)),
            "ht": np.ascontiguousarray(h[r * S:(r + 1) * S, :].T),
            "w1": np.ascontiguousarray(W1),
            "b1": np.ascontiguousarray(b1).reshape(1, -1),
            "w2": np.ascontiguousarray(W2),
            "b2": np.ascontiguousarray(b2).reshape(1, -1),
        })
    return in_maps


_RUN_KW = {}  # test harness can set {"trace": True, "tmpdir": ...}
_LAST_RESULT = [None]


def kernel(h, adj, W1, b1, W2, b2):
    from concourse.bass_utils import run_bass_kernel_spmd

    h = np.ascontiguousarray(h, dtype=np.float32)
    adj = np.ascontiguousarray(adj, dtype=np.float32)
    nc = build_graph()
    in_maps = make_in_maps(h, adj, W1, b1, W2, b2)
    res = run_bass_kernel_spmd(nc, in_maps, core_ids=list(range(NCORES)), **_RUN_KW)
    _LAST_RESULT[0] = res
    return np.concatenate([res.results[r]["out"] for r in range(NCORES)], axis=0)
